# revision 1
# baseline (speedup 1.0000x reference)
"""Multi-head causal attention (B=2, T=2048, E=768, H=12, D=64) on 8 trn2 cores.

Sharding: core c handles batch b=c//4 and heads [3g, 3g+1, 3g+2] (g=c%4).
Each core computes its 3 heads' attention plus their partial contribution to the
final projection; the host sums the 4 partials per batch.

Per-core device program (all matmul operands bf16, psum accumulation fp32;
PSUM bank budget of 8: stA0/stA1 [128,1024] (2 banks each), otl0/1/2
[128,512], bcp [128,512]):

  projections (proj_chunk(n), n=0..3): qT/kT/vT = W^T x^T for token chunk
  512n..512n+512, five 128-row weight groups [q0q1][k0k1][q2k2][v0v1][v2]
  sharing the streamed xT tiles; bias added on DVE (psum->sbuf, bf16 out).
  transposes(i): v tiles PE-transposed into v_all [128, NT, 3, 65] whose
  65th column stays 1.0 (memset once) for the fused softmax denominator.

  attention: quarter-lagged decoupled pipeline.  Quarter q covers queries
  [512q, 512q+512) with i iterating causal key blocks.  sA(q+1, q) emits
  quarter q+1's heads-0/1 QK+exp stream merged with quarter q's heads-0/1
  PV stream; sB does the same for head 2.  Because PV(q, i) only consumes
  exp results ACT finished a full quarter earlier, the in-order PE queue
  never blocks on ACT, and QKs run a quarter ahead so ACT rarely starves.
  Heads 0/1 QK go into one [128,1024] psum mega (h0 packed at 512-ln, h1
  at 512 -> one contiguous exp of width 2ln); h1 uses PE array rows
  64:127 (tile_position row packing) so h0/h1 matmuls overlap.  exp on
  ACT (scale fused), bf16 out; diagonal blocks are masked AFTER exp by
  zeroing the upper triangle with gpsimd affine_select (SBUF-only, keeps
  the mask off DVE/PSUM).  PV accumulates [O^T; l] into otl{h} via the
  [v | 1] trick.  Normalize: DVE reciprocal of the l row, PE K=1 matmul
  broadcast, ACT->SBUF copy, DVE multiply -> ot01/ot2 (bf16).

  output projection: interleaved into the quarter loop -- normalize(q)
  unlocks out rows [2048q/4...]; each quarter emits its four
  [128, 768] = [ot01; ot2]^T @ wf tiles through the bcp bank, fp32
  partial DMA'd out.  The host sums the 4 per-batch partials.

`repeat` unrolls the whole body N times in one NEFF; used by test.py to
measure per-body HW time as the slope of wall time vs N.
"""
import numpy as np

EMBED_DIM = 768
B = 2
T = 2048
N_CORES = 8
NT = T // 128           # 16 query/key tiles
SCALE = 1.0 / np.sqrt(64.0)
NEG = -1.0e9

_state = {}


def _build(repeat=1):
    import concourse.tile as tile
    from concourse import bacc, mybir
    from concourse.masks import make_identity

    F32 = mybir.dt.float32
    BF16 = mybir.dt.bfloat16

    nc = bacc.Bacc("TRN2", target_bir_lowering=False, debug=False)

    xT_d = nc.dram_tensor("xT", [EMBED_DIM, T], BF16, kind="ExternalInput").ap()
    # columns ordered [q0 q1 | k0 k1 | q2 | k2]
    wqk_d = nc.dram_tensor("wqk", [EMBED_DIM, 384], BF16, kind="ExternalInput").ap()
    wv_d = nc.dram_tensor("wv", [EMBED_DIM, 192], BF16, kind="ExternalInput").ap()
    bqk_d = nc.dram_tensor("bqk", [384, 1], F32, kind="ExternalInput").ap()
    bv_d = nc.dram_tensor("bv", [192, 1], F32, kind="ExternalInput").ap()
    wf_d = nc.dram_tensor("wf", [192, EMBED_DIM], BF16, kind="ExternalInput").ap()
    out_d = nc.dram_tensor("out_p", [T, EMBED_DIM], F32, kind="ExternalOutput").ap()

    with tile.TileContext(nc) as tc:
        with tc.tile_pool(name="const", bufs=1) as const, \
             tc.tile_pool(name="persist", bufs=1) as persist, \
             tc.tile_pool(name="sbod", bufs=1) as sbp, \
             tc.tile_pool(name="psod", bufs=1, space="PSUM") as psp:
            # ---- constants ----
            wqk_sb = const.tile([128, 6, 384], BF16)
            wv_sb = const.tile([128, 6, 192], BF16)
            nc.sync.dma_start(out=wqk_sb[:], in_=wqk_d.rearrange("(k p) c -> p k c", p=128))
            nc.gpsimd.dma_start(out=wv_sb[:], in_=wv_d.rearrange("(k p) c -> p k c", p=128))
            bqk_sb = [const.tile([128, 1], F32, name=f"bqk{m}", tag=f"bqk{m}")
                      for m in range(3)]
            for m in range(3):
                nc.sync.dma_start(out=bqk_sb[m][:], in_=bqk_d[128 * m:128 * (m + 1), :])
            bv_sb = [const.tile([128, 1], F32, name="bv0", tag="bv0"),
                     const.tile([64, 1], F32, name="bv1", tag="bv1")]
            nc.sync.dma_start(out=bv_sb[0][:], in_=bv_d[0:128, :])
            nc.sync.dma_start(out=bv_sb[1][:], in_=bv_d[128:192, :])
            wf01_sb = const.tile([128, EMBED_DIM], BF16)
            wf2_sb = const.tile([64, EMBED_DIM], BF16)
            nc.gpsimd.dma_start(out=wf01_sb[:], in_=wf_d[0:128, :])
            nc.gpsimd.dma_start(out=wf2_sb[:], in_=wf_d[128:192, :])
            ident_f = const.tile([128, 128], F32)
            make_identity(nc, ident_f)
            ident_b = const.tile([128, 128], BF16)
            nc.vector.tensor_copy(out=ident_b[:], in_=ident_f[:])
            ones_f = const.tile([128, 64], F32)
            nc.vector.memset(ones_f[:], 1.0)
            ones_r = const.tile([128, 64], mybir.dt.float32r)
            nc.vector.tensor_copy(out=ones_r[:], in_=ones_f[:])

            # ---- persistent activations ----
            qA = persist.tile([128, T], BF16)   # q0 @0:64, q1 @64:128
            kA = persist.tile([128, T], BF16)   # k0 @0:64, k1 @64:128
            qB = persist.tile([64, T], BF16)    # q2
            kB = persist.tile([64, T], BF16)    # k2
            v_all = persist.tile([128, NT, 3, 65], BF16)   # [v | 1] per head
            nc.vector.memset(v_all[:], 1.0)     # col 64 of each slot stays 1.0
            ot01 = persist.tile([128, T], BF16)  # normalized O^T heads 0 (@0) & 1 (@64)
            ot2 = persist.tile([64, T], BF16)    # head 2

            # Cross-body software pipeline: body r+1's prologue (input
            # DMA + proj chunk 0 + attention bootstrap) is emitted before
            # body r's final, PV-only quarter so ACT keeps working across
            # the body boundary.
            env = locals()
            bodies = [_make_body(nc, tc, rep, env) for rep in range(repeat)]
            bodies[0][0]()
            for rep in range(repeat):
                nxt = bodies[rep + 1][0] if rep + 1 < repeat else None
                bodies[rep][1](nxt)

    nc.compile()
    return nc


def _make_body(nc, tc, rep, env):
    """Build one body's emission closures; returns (prologue, main).
    (env KPHASES: "1", "12", or default "123")"""
    import os
    from concourse import mybir

    F32 = mybir.dt.float32
    BF16 = mybir.dt.bfloat16
    Exp = mybir.ActivationFunctionType.Exp
    MULT = mybir.AluOpType.mult
    GE = mybir.AluOpType.is_ge

    xT_d, out_d = env["xT_d"], env["out_d"]
    wqk_sb, wv_sb = env["wqk_sb"], env["wv_sb"]
    bqk_sb, bv_sb = env["bqk_sb"], env["bv_sb"]
    wf01_sb, wf2_sb = env["wf01_sb"], env["wf2_sb"]
    ident_b = env["ident_b"]
    ones_r = env["ones_r"]
    qA, kA, qB, kB = env["qA"], env["kA"], env["qB"], env["kB"]
    v_all = env["v_all"]
    ot01, ot2 = env["ot01"], env["ot2"]
    dmae = [nc.sync, nc.gpsimd]
    kphases = os.environ.get("KPHASES", "123")
    kdup = os.environ.get("KDUP", "")

    sbp, psp = env["sbp"], env["psp"]
    # ---- input DMA: xT as 6x4 chunks ----
    xT_t = [[sbp.tile([128, 512], BF16, name=f"xT{rep}_{k}_{n}",
                      tag=f"xT{k}{n}") for n in range(4)] for k in range(6)]

    def emit_xt_dma():
        for n in range(4):
            for k in range(6):
                nc.sync.dma_start(
                    out=xT_t[k][n][:],
                    in_=xT_d[128 * k:128 * (k + 1), 512 * n:512 * (n + 1)])

    gidx = [0]
    vT_sb = [sbp.tile([128, T], BF16, name=f"vT{rep}_0", tag="vT0"),
             sbp.tile([64, T], BF16, name=f"vT{rep}_1", tag="vT1")]

    def qk_group(m, n):
        # m: 0=[q0q1]->qA, 1=[k0k1]->kA, 2=[q2|k2]->qB+kB
        c0, c1 = 128 * m, 128 * (m + 1)
        ps = psp.tile([128, 512], F32, name=f"pg{rep}_{gidx[0]}",
                      tag=f"stA{gidx[0] % 2}")
        gidx[0] += 1
        for k in range(6):
            nc.tensor.matmul(ps[:], lhsT=wqk_sb[:, k, c0:c1],
                             rhs=xT_t[k][n][:], start=(k == 0), stop=(k == 5))
        nsl = slice(512 * n, 512 * (n + 1))
        if m < 2:
            dst = qA if m == 0 else kA
            nc.vector.tensor_scalar_add(out=dst[:, nsl], in0=ps[:],
                                        scalar1=bqk_sb[m][:])
        else:
            nc.vector.tensor_scalar_add(out=qB[:, nsl], in0=ps[0:64, :],
                                        scalar1=bqk_sb[2][0:64, :])
            nc.vector.tensor_scalar_add(out=kB[:, nsl], in0=ps[64:128, :],
                                        scalar1=bqk_sb[2][64:128, :])

    def v_group(m, n):
        pm = 128 if m == 0 else 64
        ps = psp.tile([128, 512], F32, name=f"pg{rep}_{gidx[0]}",
                      tag=f"stA{gidx[0] % 2}")
        gidx[0] += 1
        for k in range(6):
            nc.tensor.matmul(ps[:pm, :],
                             lhsT=wv_sb[:, k, 128 * m:128 * m + pm],
                             rhs=xT_t[k][n][:], start=(k == 0), stop=(k == 5))
        nc.vector.tensor_scalar_add(
            out=vT_sb[m][:pm, 512 * n:512 * (n + 1)],
            in0=ps[:pm, :], scalar1=bv_sb[m][:pm, :])

    def proj_chunk(n):
        qk_group(0, n)
        qk_group(1, n)
        qk_group(2, n)
        v_group(0, n)
        v_group(1, n)

    def transposes(i):
        for h in range(3):
            m, off = divmod(64 * h, 128)
            tp = psp.tile([128, 64], BF16, name=f"tp{rep}_{i}{h}",
                          tag=f"otl{h}")
            nc.tensor.transpose(
                tp[:], vT_sb[m][off:off + 64, 128 * i:128 * (i + 1)],
                ident_b[off:off + 64, off:off + 64])
            nc.vector.tensor_copy(out=v_all[:, i, h, 0:64], in_=tp[:])

    # ---- attention: quarter-lagged decoupled pipeline ----
    # Quarter q+1's QK/exp stream is merged with quarter q's PV stream.
    # By the time PE reaches PV(q, i), ACT finished exp(q, i) a whole
    # quarter ago, so the in-order PE queue never blocks on ACT, and the
    # QKs run far enough ahead that ACT never starves.
    pt01s = {}   # (q, i) -> pt01 tile
    pt2s = {}    # (q, i) -> pt2 tile
    otls = {}    # q -> otl list

    def jmaxq(q):
        return 4 * q + 3

    def geom(q, i):
        base = 512 * q
        s0 = max(base, 128 * i)
        return s0, base + 512 - s0, s0 - base   # s0, ln, co

    def qk01_exp(qq, i):
        s0, ln, _ = geom(qq, i)
        st01 = psp.tile([128, 1024], F32, name=f"st{rep}_{qq}{i}",
                        tag=f"stA{i % 2}")
        # h0 packed at [512-ln, 512); h1 at [512, 512+ln)
        for _ in range(2 if "q" in kdup else 1):
            nc.tensor.matmul(st01[:, 512 - ln:512],
                             lhsT=kA[0:64, 128 * i:128 * (i + 1)],
                             rhs=qA[0:64, s0:s0 + ln], start=True, stop=True)
            nc.tensor.matmul(st01[:, 512:512 + ln],
                             lhsT=kA[64:128, 128 * i:128 * (i + 1)],
                             rhs=qA[64:128, s0:s0 + ln],
                             start=True, stop=True)
        pt01 = sbp.tile([128, 1024], BF16, name=f"pt{rep}_{qq}{i}",
                        tag=f"pt{(qq % 2) * 3 + i % 3}")
        for _ in range(2 if "e" in kdup else 1):
            nc.scalar.activation(out=pt01[:, 512 - ln:512 + ln],
                                 in_=st01[:, 512 - ln:512 + ln],
                                 func=Exp, scale=float(SCALE))
        if "d" in kdup:   # dummy exp, no consumers: throughput probe
            dmy = sbp.tile([128, 1024], BF16, name=f"dm{rep}_{qq}{i}",
                           tag=f"dm{i % 2}")
            nc.scalar.activation(out=dmy[:, 0:2 * ln],
                                 in_=st01[:, 512 - ln:512 + ln],
                                 func=Exp, scale=float(SCALE))
        if s0 == 128 * i:
            # diagonal block: zero masked entries of the first 128 cols:
            # keep where (query c) - (key p) >= 0, else 0
            d01 = pt01[:, 512 - ln:512 + ln].rearrange(
                "p (b c) -> p b c", b=2, c=ln)[:, :, 0:128]
            nc.gpsimd.affine_select(
                out=d01, in_=d01, compare_op=GE, fill=0.0,
                base=0, channel_multiplier=-1,
                pattern=[[0, 2], [1, 128]])
        pt01s[(qq, i)] = pt01

    def pv01(pvq, i):
        _, ln, co = geom(pvq, i)
        otl = otls[pvq]
        pt01 = pt01s.pop((pvq, i))
        nd = 2 if "v" in kdup else 1   # 2x accumulation cancels in softmax
        for d in range(nd):
            st = (i == 0) and d == 0
            sp = (i == jmaxq(pvq)) and d == nd - 1
            nc.tensor.matmul(otl[0][0:65, co:512], lhsT=v_all[:, i, 0, :],
                             rhs=pt01[:, 512 - ln:512], start=st, stop=sp)
            nc.tensor.matmul(otl[1][0:65, co:512], lhsT=v_all[:, i, 1, :],
                             rhs=pt01[:, 512:512 + ln], start=st, stop=sp)

    def pv2(pvq, i):
        _, ln, co = geom(pvq, i)
        otl = otls[pvq]
        pt2, c0 = pt2s.pop((pvq, i))
        nd = 2 if "v" in kdup else 1
        for d in range(nd):
            st = (i == 0) and d == 0
            sp = (i == jmaxq(pvq)) and d == nd - 1
            nc.tensor.matmul(otl[2][0:65, co:512], lhsT=v_all[:, i, 2, :],
                             rhs=pt2[:, c0:c0 + ln], start=st, stop=sp)

    def sA(qq, pvq):
        """QK+exp for heads 0/1 of quarter qq, merged with quarter pvq's
        heads-0/1 PVs (either may be None)."""
        if pvq is not None:
            otls[pvq] = [psp.tile([128, 512], F32,
                                  name=f"otl{rep}_{pvq}{h}", tag=f"otl{h}")
                         for h in range(3)]
        n = max(jmaxq(qq) + 1 if qq is not None else 0,
                jmaxq(pvq) + 1 if pvq is not None else 0)
        for i in range(n):
            if qq is not None and i <= jmaxq(qq):
                qk01_exp(qq, i)
            if pvq is not None and i <= jmaxq(pvq):
                pv01(pvq, i)

    def sB(qq, pvq):
        """QK+exp for head 2 of quarter qq (two key blocks per psum tile
        and per exp call: block 2p at [512-ln0, 512), block 2p+1 at
        [512, 512+ln1) -> one contiguous exp), merged with pvq's head-2
        PVs.  jmax+1 = 4q+4 is always even, so blocks pair cleanly."""
        npair = (jmaxq(qq) + 1) // 2 if qq is not None else 0
        n = max(npair, jmaxq(pvq) + 1 if pvq is not None else 0)
        for step in range(n):
            if step < npair:
                i0, i1 = 2 * step, 2 * step + 1
                s00, ln0, _ = geom(qq, i0)
                s01, ln1, _ = geom(qq, i1)
                st2 = psp.tile([128, 1024], F32, name=f"s2{rep}_{qq}{step}",
                               tag=f"stA{step % 2}")
                for _ in range(2 if "q" in kdup else 1):
                    nc.tensor.matmul(st2[:, 512 - ln0:512],
                                     lhsT=kB[:, 128 * i0:128 * (i0 + 1)],
                                     rhs=qB[:, s00:s00 + ln0],
                                     start=True, stop=True)
                    nc.tensor.matmul(st2[:, 512:512 + ln1],
                                     lhsT=kB[:, 128 * i1:128 * (i1 + 1)],
                                     rhs=qB[:, s01:s01 + ln1],
                                     start=True, stop=True)
                pt2 = sbp.tile([128, 1024], BF16, name=f"p2{rep}_{qq}{step}",
                               tag=f"qt{(qq % 2) * 2 + step % 2}")
                for _ in range(2 if "e" in kdup else 1):
                    nc.scalar.activation(out=pt2[:, 512 - ln0:512 + ln1],
                                         in_=st2[:, 512 - ln0:512 + ln1],
                                         func=Exp, scale=float(SCALE))
                for (i, s0, c0) in ((i0, s00, 512 - ln0), (i1, s01, 512)):
                    if s0 == 128 * i:
                        nc.gpsimd.affine_select(
                            out=pt2[:, c0:c0 + 128], in_=pt2[:, c0:c0 + 128],
                            compare_op=GE, fill=0.0,
                            base=0, channel_multiplier=-1,
                            pattern=[[1, 128]])
                    pt2s[(qq, i)] = (pt2, c0)
            if pvq is not None and step <= jmaxq(pvq):
                pv2(pvq, step)

    def phase3_tile(i):
        # out[128i:128i+128, :] = [ot01; ot2][:, tile i].T @ wf
        fpa = psp.tile([128, 512], F32, name=f"fpa{rep}_{i}", tag="bcp")
        fpb = psp.tile([128, 256], F32, name=f"fpb{rep}_{i}",
                       tag=f"otl{i % 3}")
        for (fp, n0, n1) in [(fpa, 0, 512), (fpb, 512, 768)]:
            nc.tensor.matmul(fp[:, 0:n1 - n0],
                             lhsT=ot01[:, 128 * i:128 * (i + 1)],
                             rhs=wf01_sb[:, n0:n1], start=True, stop=False)
            nc.tensor.matmul(fp[:, 0:n1 - n0],
                             lhsT=ot2[:, 128 * i:128 * (i + 1)],
                             rhs=wf2_sb[:, n0:n1], start=False, stop=True)
        ob = sbp.tile([128, EMBED_DIM], F32, name=f"ob{rep}_{i}",
                      tag=f"ob{i % 3}")
        nc.scalar.copy(out=ob[:, 0:512], in_=fpa[:, :])
        nc.vector.tensor_copy(out=ob[:, 512:768], in_=fpb[:, :])
        dmae[i % 2].dma_start(out=out_d[128 * i:128 * (i + 1), :], in_=ob[:])

    def normalize(q):
        # rows 0:64 of otl[h] divided by row 64 (= l).  All three
        # reciprocals are issued first so the PE broadcast matmuls only
        # wait on the first one.
        otl = otls.pop(q)
        base = 512 * q
        F32R = mybir.dt.float32r
        recips, bcss = [], []
        for h in range(3):
            recip = sbp.tile([128, 512], F32R, name=f"rc{rep}_{q}{h}",
                             tag=f"rc{h}")
            with nc.allow_low_precision(reason="f32r recip for bcast"):
                nc.vector.reciprocal(out=recip[64:65, :],
                                     in_=otl[h][64:65, :])
            recips.append(recip)
        for h in range(3):
            bc = psp.tile([128, 512], F32, name=f"bc{rep}_{q}{h}",
                          tag="bcp")
            nc.tensor.matmul(bc[0:64, :], lhsT=ones_r[64:65, :],
                             rhs=recips[h][64:65, :], start=True, stop=True)
            bcs = sbp.tile([128, 512], F32, name=f"bs{rep}_{q}{h}",
                           tag=f"bs{h}")
            nc.scalar.copy(out=bcs[0:64, :], in_=bc[0:64, :])
            bcss.append(bcs)
        for h in range(3):
            ot_dst = ot2 if h == 2 else ot01
            ob_ = 64 if h == 1 else 0
            nc.vector.tensor_tensor(
                out=ot_dst[ob_:ob_ + 64, base:base + 512],
                in0=otl[h][0:64, :], in1=bcss[h][0:64, :], op=MULT)

    # ---- emission closures ----
    def prologue():
        # input DMA + first proj chunk + attention bootstrap.  No
        # transposes here: they write v_all, which the PREVIOUS body's
        # final-quarter PVs still read when this runs pipelined.
        emit_xt_dma()
        proj_chunk(0)
        if "2" in kphases:
            sA(0, None)
            sB(0, None)

    def main(next_prologue=None):
        for i in range(4):
            transposes(i)
        if "2" in kphases:
            for q in range(4):
                if q == 3 and next_prologue is not None:
                    next_prologue()
                if q < 3:
                    proj_chunk(q + 1)
                nxt = q + 1 if q < 3 else None
                sA(nxt, q)
                sB(nxt, q)
                normalize(q)
                if q < 3:
                    for i in range(4 * q + 4, 4 * q + 8):
                        transposes(i)
                if "3" in kphases:
                    for i in range(4 * q, 4 * q + 4):
                        phase3_tile(i)
        else:
            for q in range(1, 4):
                proj_chunk(q)
            for i in range(4, 16):
                transposes(i)

        # degenerate KPHASES modes: keep the output written / ot read
        if "2" not in kphases:
            ob0 = sbp.tile([128, EMBED_DIM], F32, name=f"ob{rep}_d", tag="ob0")
            nc.vector.memset(ob0[:], 0.0)
            dmae[0].dma_start(out=out_d[0:128, :], in_=ob0[:])
        elif "3" not in kphases:
            phase3_tile(0)

    return prologue, main


def _prep_inputs(x, w_qkv, b_qkv, w_final):
    """Build the 8 per-core input maps from the full inputs."""
    x = np.asarray(x, dtype=np.float32)
    w_qkv = np.asarray(w_qkv, dtype=np.float32)
    b_qkv = np.asarray(b_qkv, dtype=np.float32)
    w_final = np.asarray(w_final, dtype=np.float32)
    E = EMBED_DIM

    in_maps = []
    for c in range(N_CORES):
        b = c // 4
        g = c % 4
        heads = [3 * g, 3 * g + 1, 3 * g + 2]
        hr = [np.arange(64 * h, 64 * h + 64) for h in heads]
        # [q0 q1 | k0 k1 | q2 | k2]
        rows_qk = np.concatenate([hr[0], hr[1], E + hr[0], E + hr[1], hr[2], E + hr[2]])
        rows_v = np.concatenate(hr) + 2 * E
        import ml_dtypes
        bf16 = ml_dtypes.bfloat16
        xT = np.ascontiguousarray(x[b].T).astype(bf16)          # [768, 2048]
        wqk = np.ascontiguousarray(w_qkv[rows_qk].T).astype(bf16)   # [768, 384]
        wv = np.ascontiguousarray(w_qkv[rows_v].T).astype(bf16)     # [768, 192]
        bqk = np.ascontiguousarray(b_qkv[rows_qk][:, None])
        bv = np.ascontiguousarray(b_qkv[rows_v][:, None])
        wf = np.ascontiguousarray(w_final[:, np.concatenate(hr)].T).astype(bf16)
        in_maps.append({"xT": xT, "wqk": wqk, "wv": wv, "bqk": bqk, "bv": bv,
                        "wf": wf})
    return in_maps


def kernel(x, w_qkv, b_qkv, w_final, _trace=False):
    from concourse.bass_utils import run_bass_kernel_spmd

    if "nc" not in _state:
        _state["nc"] = _build()
    nc = _state["nc"]

    in_maps = _prep_inputs(x, w_qkv, b_qkv, w_final)
    res = run_bass_kernel_spmd(nc, in_maps, list(range(N_CORES)), trace=_trace)
    _state["last_result"] = res

    out = np.empty((B, T, EMBED_DIM), dtype=np.float32)
    for b in range(B):
        acc = np.zeros((T, EMBED_DIM), dtype=np.float64)
        for g in range(4):
            acc += res.results[4 * b + g]["out_p"].astype(np.float64)
        out[b] = acc.astype(np.float32)
    return out

